# revision 26
# baseline (speedup 1.0000x reference)
"""Dynamic tree drafting loop on 8 trn2 NeuronCores.

Strategy: tensor-parallel LM head over vocab (V=32000 -> 4000 per core).
Each core holds its w_lm shard in SBUF (16MB, loaded once), computes
logits for all frontier rows every layer, does an exact per-shard top-10
(DVE max8/max_index/match_replace), and AllGathers (candidate values,
global token ids, sum-of-exp) per layer.  Every core then redundantly
combines shards: global per-row top-10, log-softmax adjust, tree
bookkeeping (history scatter, frontier re-select, parent tracking),
embedding gather (indirect DMA) for the next layer.  The final
top-T=63 re-sample over the [8,310] history is done on host.
"""

import numpy as np

import concourse.bass as bass
import concourse.mybir as mybir
from concourse import bacc, tile
from concourse.masks import make_identity

F32 = mybir.dt.float32
U32 = mybir.dt.uint32
I32 = mybir.dt.int32
ALU = mybir.AluOpType
ACT = mybir.ActivationFunctionType
AX = mybir.AxisListType

K = 10          # dynamic_tree_max_topK
L = 4           # max_draft_len
T = 63          # max_total_draft_tokens
B, V, D = 8, 32000, 1024
NCORES = 8
VS = V // NCORES          # 4000 vocab per core
NCH = 8                   # psum chunks per shard
CH = VS // NCH            # 500
ROWS = B * K              # 80 frontier rows at layers >= 1
NEG = -1e30


def build_nc():
    nc = bacc.Bacc(trn_type="TRN2", target_bir_lowering=False, num_devices=NCORES)

    # ---- kernel I/O ----
    w_sh = nc.declare_dram_parameter("w_sh", [D, VS], F32, isOutput=False)
    embed_h = nc.declare_dram_parameter("embed", [V, D], F32, isOutput=False)
    x0t_h = nc.declare_dram_parameter("x0t", [128, 8 * B], F32, isOutput=False)
    offs_h = nc.declare_dram_parameter("offs", [128, 1], F32, isOutput=False)
    hist_sc_o = nc.declare_dram_parameter("hist_sc", [B, 310], F32, isOutput=True)
    hist_tok_o = nc.declare_dram_parameter("hist_tok", [B, 310], F32, isOutput=True)
    parents_o = nc.declare_dram_parameter("parents", [B, 31], F32, isOutput=True)

    RG = [list(range(NCORES))]

    with tile.TileContext(nc) as tc:
        with (
            tc.tile_pool(name="const", bufs=1) as const,
            tc.tile_pool(name="wpool", bufs=1) as wpool,
            tc.tile_pool(name="work", bufs=1) as work,
            tc.tile_pool(name="psum", bufs=1, space="PSUM") as psum_p,
            tc.tile_pool(name="dram", bufs=2, space="DRAM") as dram_p,
        ):
            # ---- persistent / constant tiles ----
            iota_c = const.tile([128, 100], F32, tag="iota")
            nc.gpsimd.iota(iota_c[:, :], pattern=[[1, 100]], base=0,
                           channel_multiplier=0,
                           allow_small_or_imprecise_dtypes=True)
            ident = const.tile([128, 128], F32, tag="ident")
            make_identity(nc, ident[:, :])
            offs_sb = const.tile([128, 1], F32, tag="offs")
            nc.sync.dma_start(out=offs_sb[:, :], in_=offs_h[:, :])
            x0t_sb = const.tile([128, 8 * B], F32, tag="x0t")
            nc.sync.dma_start(out=x0t_sb[:, :], in_=x0t_h[:, :])

            hist_sc_sb = const.tile([128, 310], F32, tag="hsc")
            hist_tok_sb = const.tile([128, 310], F32, tag="htok")
            parents_sb = const.tile([128, 31], F32, tag="par")
            R80 = const.tile([128, 1], F32, tag="r80")
            nc.vector.memset(R80[:, :], 0.0)
            sel_pos = const.tile([128, K], F32, tag="selpos")

            # ---- w shard -> SBUF, one 2MB DMA per 128-row k-slab ----
            w_sb = []
            for k in range(8):
                wk = wpool.tile([128, VS], F32, tag=f"w{k}")
                nc.sync.dma_start(out=wk[:, :], in_=w_sh[128 * k:128 * (k + 1), :])
                w_sb.append(wk)

            xT_cur = None  # SBUF tile [128, 80*8] for layers >= 1

            for l in range(L):
                rows = B if l == 0 else ROWS
                # ---------- matmul: logits for this layer's rows ----------
                ps = psum_p.tile([128, 4096], F32, tag="ps")
                for c in range(NCH):
                    for k in range(8):
                        if l == 0:
                            lhsT = x0t_sb[:, B * k:B * (k + 1)]
                        else:
                            lhsT = xT_cur[:, ROWS * k:ROWS * (k + 1)]
                        nc.tensor.matmul(
                            ps[:rows, 512 * c:512 * c + CH],
                            lhsT=lhsT,
                            rhs=w_sb[k][:, CH * c:CH * (c + 1)],
                            start=(k == 0),
                            stop=(k == 7),
                        )

                # ---------- PSUM -> contiguous SBUF logits ----------
                logits_sb = work.tile([128, VS], F32, tag="logits")
                for c in range(NCH):
                    nc.vector.tensor_copy(logits_sb[:rows, CH * c:CH * (c + 1)],
                                          ps[:rows, 512 * c:512 * c + CH])

                # ---------- sum(exp(logit)) per row (for log-softmax) ----------
                sums = work.tile([128, NCH], F32, tag="sums")
                junk_exp = work.tile([128, CH], F32, tag="jexp")
                for c in range(NCH):
                    nc.scalar.activation(
                        out=junk_exp[:rows, :CH],
                        in_=logits_sb[:rows, CH * c:CH * (c + 1)],
                        func=ACT.Exp,
                        accum_out=sums[:rows, c:c + 1],
                    )
                sumexp = work.tile([128, 1], F32, tag="sumexp")
                nc.vector.reduce_sum(out=sumexp[:rows, :], in_=sums[:rows, :NCH],
                                     axis=AX.X)

                # ---------- exact per-shard top-10 over 4000 ----------
                m1 = work.tile([128, 8], F32, tag="m1")
                i1 = work.tile([128, 8], U32, tag="i1")
                m2 = work.tile([128, 8], F32, tag="m2")
                i2 = work.tile([128, 8], U32, tag="i2")
                logits2 = work.tile([128, VS], F32, tag="logits2")
                lv = logits_sb[:rows, :VS]
                nc.vector.max(m1[:rows, :], lv)
                nc.vector.max_index(i1[:rows, :], m1[:rows, :], lv)
                nc.vector.match_replace(out=logits2[:rows, :VS],
                                        in_to_replace=m1[:rows, :],
                                        in_values=lv, imm_value=NEG)
                nc.vector.max(m2[:rows, :], logits2[:rows, :VS])
                nc.vector.max_index(i2[:rows, :], m2[:rows, :], logits2[:rows, :VS])

                # ---------- candidate block [80, 24] & AllGather ----------
                blk = work.tile([128, 24], F32, tag="blk")
                nc.vector.memset(blk[:80, :24], 0.0)
                nc.vector.tensor_copy(blk[:rows, 0:8], m1[:rows, :])
                nc.vector.tensor_copy(blk[:rows, 8:10], m2[:rows, 0:2])
                idxf = work.tile([128, K], F32, tag="idxf")
                nc.vector.tensor_copy(idxf[:rows, 0:8], i1[:rows, :])
                nc.vector.tensor_copy(idxf[:rows, 8:10], i2[:rows, 0:2])
                nc.vector.tensor_tensor(blk[:rows, 10:20], idxf[:rows, :K],
                                        offs_sb[:rows, :].to_broadcast([rows, K]),
                                        op=ALU.add)
                nc.vector.tensor_copy(blk[:rows, 20:21], sumexp[:rows, :])

                cc_in = dram_p.tile([80, 24], F32, tag="ccin", bufs=4)
                cc_out = dram_p.tile([80 * NCORES, 24], F32, tag="ccout",
                                     addr_space="Shared", bufs=4)
                nc.sync.dma_start(out=cc_in[:80, :], in_=blk[:80, :24])
                nc.gpsimd.collective_compute(
                    "AllGather", ALU.bypass, replica_groups=RG,
                    ins=[cc_in[:, :].opt()], outs=[cc_out[:, :].opt()],
                )

                # ---------- readback + cross-shard combine (all cores) ----------
                # contiguous per-kind tiles: values, global ids, per-shard sums
                gsrc = cc_out[:, :].rearrange("(s r) e -> r s e", s=NCORES)[:rows]
                Vg = work.tile([128, 80], F32, tag="vg")
                Ig = work.tile([128, 80], F32, tag="ig")
                Sg = work.tile([128, NCORES], F32, tag="sg")
                nc.sync.dma_start(
                    out=Vg[:rows, :80].rearrange("r (s e) -> r s e", e=K),
                    in_=gsrc[:, :, 0:K])
                nc.sync.dma_start(
                    out=Ig[:rows, :80].rearrange("r (s e) -> r s e", e=K),
                    in_=gsrc[:, :, 10:10 + K])
                nc.sync.dma_start(
                    out=Sg[:rows, :NCORES].rearrange("r (s e) -> r s e", e=1),
                    in_=gsrc[:, :, 20:21])

                sumall = work.tile([128, 1], F32, tag="sumall")
                nc.vector.reduce_sum(out=sumall[:rows, :], in_=Sg[:rows, :NCORES],
                                     axis=AX.X)
                Lt = work.tile([128, 1], F32, tag="lt")
                nc.scalar.activation(out=Lt[:rows, :], in_=sumall[:rows, :],
                                     func=ACT.Ln)
                adj = work.tile([128, 1], F32, tag="adj")
                nc.vector.tensor_sub(adj[:rows, :], R80[:rows, :], Lt[:rows, :])

                c1 = work.tile([128, 8], F32, tag="c1")
                ci1 = work.tile([128, 8], U32, tag="ci1")
                c2 = work.tile([128, 8], F32, tag="c2")
                ci2 = work.tile([128, 8], U32, tag="ci2")
                Vz = work.tile([128, 80], F32, tag="vz")
                Vv = Vg[:rows, :80]
                nc.vector.max(c1[:rows, :], Vv)
                nc.vector.max_index(ci1[:rows, :], c1[:rows, :], Vv)
                nc.vector.match_replace(out=Vz[:rows, :80], in_to_replace=c1[:rows, :],
                                        in_values=Vv, imm_value=NEG)
                nc.vector.max(c2[:rows, :], Vz[:rows, :80])
                nc.vector.max_index(ci2[:rows, :], c2[:rows, :], Vz[:rows, :80])

                v10 = work.tile([128, K], F32, tag="v10")
                p10 = work.tile([128, K], F32, tag="p10")
                nc.vector.tensor_copy(v10[:rows, 0:8], c1[:rows, :])
                nc.vector.tensor_copy(v10[:rows, 8:10], c2[:rows, 0:2])
                nc.vector.tensor_copy(p10[:rows, 0:8], ci1[:rows, :])
                nc.vector.tensor_copy(p10[:rows, 8:10], ci2[:rows, 0:2])

                # gather token ids of the winning candidates:
                # tok10[r,j] = Ig[r, p10[r,j]]
                tok10 = work.tile([128, K], F32, tag="tok10")
                junkg = work.tile([128, 80], F32, tag="junkg")
                for j in range(K):
                    nc.vector.scalar_tensor_tensor(
                        out=junkg[:rows, :80], in0=iota_c[:rows, :80],
                        scalar=p10[:rows, j:j + 1], in1=Ig[:rows, :80],
                        op0=ALU.is_equal, op1=ALU.mult,
                        accum_out=tok10[:rows, j:j + 1],
                    )

                # cumulative path scores of the 10 winners (descending)
                comb10 = work.tile([128, K], F32, tag="comb10")
                nc.vector.tensor_tensor(comb10[:rows, :K], v10[:rows, :K],
                                        adj[:rows, :].to_broadcast([rows, K]),
                                        op=ALU.add)

                if l == 0:
                    # history block 0, frontier = the 10 winners per sequence
                    nc.vector.tensor_copy(hist_sc_sb[:B, 0:K], comb10[:B, :K])
                    nc.vector.tensor_copy(hist_tok_sb[:B, 0:K], tok10[:B, :K])
                    nc.vector.tensor_copy(sel_pos[:B, :K], iota_c[:B, :K])
                    nc.vector.memset(parents_sb[:B, 0:1], -1.0)
                    cur_scores_v = comb10[:B, :K]
                    cur_tokens_v = tok10[:B, :K]
                else:
                    # fold [80,10] row-major into [8, 100] per sequence
                    combb = work.tile([128, 100], F32, tag="combb")
                    tokb = work.tile([128, 100], F32, tag="tokb")
                    nc.sync.dma_start(
                        out=combb[:B, :100].rearrange("b (k e) -> b k e", e=K),
                        in_=comb10[:ROWS, :K],
                    )
                    nc.sync.dma_start(
                        out=tokb[:B, :100].rearrange("b (k e) -> b k e", e=K),
                        in_=tok10[:ROWS, :K],
                    )
                    comb_b = combb[:B, :100]
                    tok_b = tokb[:B, :100]

                    # history blocks for this layer
                    hoff = K + 100 * (l - 1)
                    nc.vector.tensor_copy(hist_sc_sb[:B, hoff:hoff + 100], comb_b)
                    nc.vector.tensor_copy(hist_tok_sb[:B, hoff:hoff + 100], tok_b)

                    # select new frontier: top-10 of the 100 comb scores
                    d1 = work.tile([128, 8], F32, tag="d1")
                    e1 = work.tile([128, 8], U32, tag="e1")
                    d2 = work.tile([128, 8], F32, tag="d2")
                    e2 = work.tile([128, 8], U32, tag="e2")
                    cz = work.tile([128, 100], F32, tag="cz")
                    nc.vector.max(d1[:B, :], comb_b)
                    nc.vector.max_index(e1[:B, :], d1[:B, :], comb_b)
                    nc.vector.match_replace(out=cz[:B, :100], in_to_replace=d1[:B, :],
                                            in_values=comb_b, imm_value=NEG)
                    nc.vector.max(d2[:B, :], cz[:B, :100])
                    nc.vector.max_index(e2[:B, :], d2[:B, :], cz[:B, :100])

                    cur_sc = work.tile([128, K], F32, tag="cursc")
                    tix = work.tile([128, K], F32, tag="tix")
                    nc.vector.tensor_copy(cur_sc[:B, 0:8], d1[:B, :])
                    nc.vector.tensor_copy(cur_sc[:B, 8:10], d2[:B, 0:2])
                    nc.vector.tensor_copy(tix[:B, 0:8], e1[:B, :])
                    nc.vector.tensor_copy(tix[:B, 8:10], e2[:B, 0:2])

                    # gather frontier tokens + parent positions
                    cur_tok = work.tile([128, K], F32, tag="curtok")
                    junk2 = work.tile([128, 100], F32, tag="junk2")
                    for j in range(K):
                        nc.vector.scalar_tensor_tensor(
                            out=junk2[:B, :100], in0=iota_c[:B, :100],
                            scalar=tix[:B, j:j + 1], in1=tok_b,
                            op0=ALU.is_equal, op1=ALU.mult,
                            accum_out=cur_tok[:B, j:j + 1],
                        )
                    # sel_exp[b, 10k+j] = sel_pos[b, k]
                    sel_exp = work.tile([128, 100], F32, tag="selexp")
                    nc.vector.tensor_copy(
                        sel_exp[:B, :100].rearrange("b (k e) -> b k e", e=K),
                        sel_pos[:B, :K].unsqueeze(2).broadcast_to([B, K, K]),
                    )
                    par10 = work.tile([128, K], F32, tag="par10")
                    for j in range(K):
                        nc.vector.scalar_tensor_tensor(
                            out=junk2[:B, :100], in0=iota_c[:B, :100],
                            scalar=tix[:B, j:j + 1],
                            in1=sel_exp[:B, :100],
                            op0=ALU.is_equal, op1=ALU.mult,
                            accum_out=par10[:B, j:j + 1],
                        )
                    nc.vector.tensor_copy(parents_sb[:B, 1 + K * (l - 1):1 + K * l],
                                          par10[:B, :K])
                    # sel_pos <- 10 + 100*(l-1) + tix
                    nc.vector.tensor_scalar(
                        out=sel_pos[:B, :K], in0=tix[:B, :K],
                        scalar1=float(K + 100 * (l - 1)), scalar2=None,
                        op0=ALU.add,
                    )
                    cur_scores_v = cur_sc[:B, :K]
                    cur_tokens_v = cur_tok[:B, :K]

                if l < L - 1:
                    # fold the new frontier for the next layer:
                    # R80[10b+k] = cur_scores[b,k];  tok_off[10b+k] = token id
                    nc.sync.dma_start(out=R80[:ROWS, :1], in_=cur_scores_v)
                    ctok_i = work.tile([128, K], I32, tag="ctoki")
                    nc.vector.tensor_copy(ctok_i[:B, :K], cur_tokens_v)
                    tok_off = work.tile([128, 1], I32, tag="tokoff")
                    nc.sync.dma_start(out=tok_off[:ROWS, :1], in_=ctok_i[:B, :K])

                    # gather the 80 embedding rows -> x [80, 1024]
                    x_sb = work.tile([128, D], F32, tag="xsb", bufs=3)
                    nc.gpsimd.indirect_dma_start(
                        out=x_sb[:ROWS, :D],
                        out_offset=None,
                        in_=embed_h[:, :],
                        in_offset=bass.IndirectOffsetOnAxis(
                            ap=tok_off[:ROWS, :1], axis=0),
                    )
                    # transpose to xT [128, 8*80] via PE
                    xT_next = work.tile([128, ROWS * 8], F32, tag="xt", bufs=3)
                    ps_tr = psum_p.tile([128, 4096], F32, tag="ps")
                    for k in range(8):
                        nc.tensor.transpose(
                            out=ps_tr[:, 512 * k:512 * k + ROWS],
                            in_=x_sb[:ROWS, 128 * k:128 * (k + 1)],
                            identity=ident[:ROWS, :ROWS],
                        )
                        nc.vector.tensor_copy(
                            xT_next[:, ROWS * k:ROWS * (k + 1)],
                            ps_tr[:, 512 * k:512 * k + ROWS])
                    xT_cur = xT_next

            # ---- outputs ----
            nc.sync.dma_start(out=hist_sc_o[:, :], in_=hist_sc_sb[:B, :310])
            nc.sync.dma_start(out=hist_tok_o[:, :], in_=hist_tok_sb[:B, :310])
            nc.sync.dma_start(out=parents_o[:, :], in_=parents_sb[:B, :31])

    nc.finalize()
    return nc


def make_in_maps(input_ids, embed, w_lm):
    """Build the 8 per-core input dicts from the full problem inputs."""
    input_ids = np.asarray(input_ids)
    embed = np.ascontiguousarray(np.asarray(embed, dtype=np.float32))
    w_lm = np.asarray(w_lm, dtype=np.float32)

    x0 = embed[input_ids]                      # [8, 1024]
    x0t = np.ascontiguousarray(
        x0.T.reshape(8, 128, B).transpose(1, 0, 2).reshape(128, 8 * B)
    ).astype(np.float32)

    in_maps = []
    for s in range(NCORES):
        in_maps.append({
            "w_sh": np.ascontiguousarray(w_lm[:, s * VS:(s + 1) * VS]),
            "embed": embed,
            "x0t": x0t,
            "offs": np.full((128, 1), s * VS, dtype=np.float32),
        })
    return in_maps


def finalize(hist_sc, hist_tok, parents):
    """Host-side final top-T resample (matches lax.top_k tie semantics)."""
    hist_sc = np.array(hist_sc)
    hist_tok = np.array(hist_tok)
    # Tie repair: the device orders each row's top-10 by raw logit, but the
    # reference orders by the (coarser) f32 log-softmax value with ties
    # broken by vocab index.  Where our comb scores collide exactly, reorder
    # the run by token id ascending, within each 10-entry history block.
    nblk = hist_sc.shape[1] // K
    for b in range(hist_sc.shape[0]):
        for blk in range(nblk):
            s = slice(blk * K, (blk + 1) * K)
            sc = hist_sc[b, s]
            tk = hist_tok[b, s]
            if np.unique(sc).size < K:
                o = np.lexsort((tk, -sc))
                hist_sc[b, s] = sc[o]
                hist_tok[b, s] = tk[o]
    order = np.argsort(-hist_sc, axis=1, kind="stable")[:, :T]
    final_sc = np.take_along_axis(hist_sc, order, axis=1).astype(np.float32)
    toks = np.take_along_axis(hist_tok, order, axis=1)
    new_draft_tokens = np.rint(toks).astype(np.int32).T
    topk_score_indices = order.astype(np.int32)
    parent_buffer = np.rint(parents).astype(np.int32)
    return new_draft_tokens, final_sc, topk_score_indices, parent_buffer


_NC_CACHE = None


def kernel(input_ids, embed, w_lm):
    global _NC_CACHE
    from concourse.bass_utils import run_bass_kernel_spmd

    if _NC_CACHE is None:
        _NC_CACHE = build_nc()
    nc = _NC_CACHE
    in_maps = make_in_maps(input_ids, embed, w_lm)
    res = run_bass_kernel_spmd(nc, in_maps, core_ids=list(range(NCORES)))
    out = res.results[0]
    return finalize(out["hist_sc"], out["hist_tok"], out["parents"])


# revision 32
# speedup vs baseline: 1.0657x; 1.0657x over previous
"""Dynamic tree drafting loop on 8 trn2 NeuronCores.

Strategy: tensor-parallel LM head over vocab (V=32000 -> 4000 per core).
Each core holds its w_lm shard in SBUF (16MB, loaded once), computes
logits for all frontier rows every layer, does an exact per-shard top-10
(DVE max8/max_index/match_replace), and AllGathers (candidate values,
global token ids, sum-of-exp) per layer.  Every core then redundantly
combines shards: global per-row top-10, log-softmax adjust, tree
bookkeeping (history scatter, frontier re-select, parent tracking),
embedding gather (indirect DMA) for the next layer.  The final
top-T=63 re-sample over the [8,310] history is done on host.
"""

import numpy as np

import concourse.bass as bass
import concourse.mybir as mybir
from concourse import bacc, tile
from concourse.masks import make_identity

F32 = mybir.dt.float32
U32 = mybir.dt.uint32
I32 = mybir.dt.int32
ALU = mybir.AluOpType
ACT = mybir.ActivationFunctionType
AX = mybir.AxisListType

K = 10          # dynamic_tree_max_topK
L = 4           # max_draft_len
T = 63          # max_total_draft_tokens
B, V, D = 8, 32000, 1024
NCORES = 8
VS = V // NCORES          # 4000 vocab per core
NCH = 8                   # psum chunks per shard
CH = VS // NCH            # 500
ROWS = B * K              # 80 frontier rows at layers >= 1
NEG = -1e30


def build_nc():
    nc = bacc.Bacc(trn_type="TRN2", target_bir_lowering=False, num_devices=NCORES)

    # ---- kernel I/O ----
    w_sh = nc.declare_dram_parameter("w_sh", [D, VS], F32, isOutput=False)
    embed_h = nc.declare_dram_parameter("embed", [V, D], F32, isOutput=False)
    x0t_h = nc.declare_dram_parameter("x0t", [128, 8 * B], F32, isOutput=False)
    offs_h = nc.declare_dram_parameter("offs", [128, 1], F32, isOutput=False)
    hist_sc_o = nc.declare_dram_parameter("hist_sc", [B, 310], F32, isOutput=True)
    hist_tok_o = nc.declare_dram_parameter("hist_tok", [B, 310], F32, isOutput=True)
    parents_o = nc.declare_dram_parameter("parents", [B, 31], F32, isOutput=True)

    RG = [list(range(NCORES))]

    with tile.TileContext(nc) as tc:
        with (
            tc.tile_pool(name="const", bufs=1) as const,
            tc.tile_pool(name="wpool", bufs=1) as wpool,
            tc.tile_pool(name="work", bufs=1) as work,
            tc.tile_pool(name="psum", bufs=1, space="PSUM") as psum_p,
            tc.tile_pool(name="dram", bufs=2, space="DRAM") as dram_p,
        ):
            # ---- persistent / constant tiles ----
            iota_c = const.tile([128, 100], F32, tag="iota")
            nc.gpsimd.iota(iota_c[:, :], pattern=[[1, 100]], base=0,
                           channel_multiplier=0,
                           allow_small_or_imprecise_dtypes=True)
            # choff[10c+j] = 500*c : per-chunk vocab base within the shard
            choff = const.tile([128, 80], F32, tag="choff")
            nc.gpsimd.iota(choff[:, :], pattern=[[CH, NCH], [0, K]], base=0,
                           channel_multiplier=0,
                           allow_small_or_imprecise_dtypes=True)
            ident = const.tile([128, 128], F32, tag="ident")
            make_identity(nc, ident[:, :])
            offs_sb = const.tile([128, 1], F32, tag="offs")
            nc.sync.dma_start(out=offs_sb[:, :], in_=offs_h[:, :])
            x0t_sb = const.tile([128, 8 * B], F32, tag="x0t")
            nc.sync.dma_start(out=x0t_sb[:, :], in_=x0t_h[:, :])

            hist_sc_sb = const.tile([128, 310], F32, tag="hsc")
            hist_tok_sb = const.tile([128, 310], F32, tag="htok")
            parents_sb = const.tile([128, 31], F32, tag="par")
            R80 = const.tile([128, 1], F32, tag="r80")
            nc.vector.memset(R80[:, :], 0.0)
            sel_pos = const.tile([128, K], F32, tag="selpos")

            # ---- w shard -> SBUF, one 2MB DMA per 128-row k-slab ----
            w_sb = []
            for k in range(8):
                wk = wpool.tile([128, VS], F32, tag=f"w{k}")
                nc.sync.dma_start(out=wk[:, :], in_=w_sh[128 * k:128 * (k + 1), :])
                w_sb.append(wk)

            xT_cur = None  # SBUF tile [128, 80*8] for layers >= 1

            for l in range(L):
                rows = B if l == 0 else ROWS
                # ---------- matmul + per-chunk top-10, chunk-pipelined ------
                # Each 500-wide psum chunk is top-k'ed on DVE (reading PSUM
                # directly) and exp-summed on ACT while PE works on the next
                # chunk, so only the final chunk's reduction is exposed.
                ps = psum_p.tile([128, 4096], F32, tag="ps")
                sums = work.tile([128, NCH], F32, tag="sums")
                junk_exp = work.tile([128, CH], F32, tag="jexp")
                chunkv = work.tile([128, 80], F32, tag="chv")
                chunki = work.tile([128, 80], F32, tag="chi")
                chunkz = work.tile([128, CH], F32, tag="chz")
                mc1 = work.tile([128, 8], F32, tag="mc1")
                mc2 = work.tile([128, 8], F32, tag="mc2")
                ic1 = work.tile([128, 8], U32, tag="ic1")
                ic2 = work.tile([128, 8], U32, tag="ic2")
                for c in range(NCH):
                    for k in range(8):
                        if l == 0:
                            lhsT = x0t_sb[:, B * k:B * (k + 1)]
                        else:
                            lhsT = xT_cur[:, ROWS * k:ROWS * (k + 1)]
                        nc.tensor.matmul(
                            ps[:rows, 512 * c:512 * c + CH],
                            lhsT=lhsT,
                            rhs=w_sb[k][:, CH * c:CH * (c + 1)],
                            start=(k == 0),
                            stop=(k == 7),
                        )
                    pvc = ps[:rows, 512 * c:512 * c + CH]
                    nc.scalar.activation(
                        out=junk_exp[:rows, :CH], in_=pvc, func=ACT.Exp,
                        accum_out=sums[:rows, c:c + 1],
                    )
                    nc.vector.max(mc1[:rows, :], pvc)
                    nc.vector.max_index(ic1[:rows, :], mc1[:rows, :], pvc)
                    nc.vector.match_replace(out=chunkz[:rows, :CH],
                                            in_to_replace=mc1[:rows, :],
                                            in_values=pvc, imm_value=NEG)
                    nc.vector.max(mc2[:rows, :], chunkz[:rows, :CH])
                    nc.vector.max_index(ic2[:rows, :], mc2[:rows, :],
                                        chunkz[:rows, :CH])
                    nc.vector.tensor_copy(chunkv[:rows, 10 * c:10 * c + 8],
                                          mc1[:rows, :])
                    nc.vector.tensor_copy(chunkv[:rows, 10 * c + 8:10 * c + 10],
                                          mc2[:rows, 0:2])
                    nc.vector.tensor_copy(chunki[:rows, 10 * c:10 * c + 8],
                                          ic1[:rows, :])
                    nc.vector.tensor_copy(chunki[:rows, 10 * c + 8:10 * c + 10],
                                          ic2[:rows, 0:2])

                sumexp = work.tile([128, 1], F32, tag="sumexp")
                nc.vector.reduce_sum(out=sumexp[:rows, :], in_=sums[:rows, :NCH],
                                     axis=AX.X)
                # globalize candidate ids: + 500*chunk + shard offset
                nc.vector.tensor_tensor(chunki[:rows, :80], chunki[:rows, :80],
                                        choff[:rows, :80], op=ALU.add)
                nc.vector.tensor_tensor(chunki[:rows, :80], chunki[:rows, :80],
                                        offs_sb[:rows, :].to_broadcast([rows, 80]),
                                        op=ALU.add)

                # ---------- shard-level top-10 of the 80 chunk candidates ----
                sm1 = work.tile([128, 8], F32, tag="sm1")
                sm2 = work.tile([128, 8], F32, tag="sm2")
                sz = work.tile([128, 80], F32, tag="sz")
                nc.vector.max(sm1[:rows, :], chunkv[:rows, :80])
                nc.vector.match_replace(out=sz[:rows, :80],
                                        in_to_replace=sm1[:rows, :],
                                        in_values=chunkv[:rows, :80],
                                        imm_value=NEG)
                nc.vector.max(sm2[:rows, :], sz[:rows, :80])
                v10s = work.tile([128, K], F32, tag="v10s")
                nc.vector.tensor_copy(v10s[:rows, 0:8], sm1[:rows, :])
                nc.vector.tensor_copy(v10s[:rows, 8:10], sm2[:rows, 0:2])
                tok10s = work.tile([128, K], F32, tag="tok10s")
                junkg = work.tile([128, 80], F32, tag="junkg")
                for j in range(K):
                    nc.vector.scalar_tensor_tensor(
                        out=junkg[:rows, :80], in0=chunkv[:rows, :80],
                        scalar=v10s[:rows, j:j + 1], in1=chunki[:rows, :80],
                        op0=ALU.is_equal, op1=ALU.mult,
                        accum_out=tok10s[:rows, j:j + 1],
                    )

                # ---------- candidate block [80, 24] & AllGather ----------
                blk = work.tile([128, 24], F32, tag="blk")
                nc.vector.memset(blk[:80, :24], 0.0)
                nc.vector.tensor_copy(blk[:rows, 0:K], v10s[:rows, :K])
                nc.vector.tensor_copy(blk[:rows, K:2 * K], tok10s[:rows, :K])
                nc.vector.tensor_copy(blk[:rows, 20:21], sumexp[:rows, :])

                cc_in = dram_p.tile([80, 24], F32, tag="ccin", bufs=4)
                cc_out = dram_p.tile([80 * NCORES, 24], F32, tag="ccout",
                                     addr_space="Shared", bufs=4)
                nc.sync.dma_start(out=cc_in[:80, :], in_=blk[:80, :24])
                nc.gpsimd.collective_compute(
                    "AllGather", ALU.bypass, replica_groups=RG,
                    ins=[cc_in[:, :].opt()], outs=[cc_out[:, :].opt()],
                )

                # ---------- readback + cross-shard combine (all cores) ----------
                gat = work.tile([128, 24 * NCORES], F32, tag="gat")
                gsrc = cc_out[:, :].rearrange("(s r) e -> r s e", s=NCORES)[:rows]
                nc.sync.dma_start(
                    out=gat[:rows, :24 * NCORES].rearrange("r (s e) -> r s e", e=24),
                    in_=gsrc)
                g3 = gat[:rows, :24 * NCORES].rearrange("r (s e) -> r s e", e=24)
                Vv = g3[:, :, 0:K]            # [rows, 8, 10] candidate values
                Iv = g3[:, :, K:2 * K]        # [rows, 8, 10] global token ids
                Sv = g3[:, :, 20:21]          # [rows, 8, 1] per-shard sumexp

                sumall = work.tile([128, 1], F32, tag="sumall")
                nc.vector.reduce_sum(out=sumall[:rows, :], in_=Sv, axis=AX.XY)
                Lt = work.tile([128, 1], F32, tag="lt")
                nc.scalar.activation(out=Lt[:rows, :], in_=sumall[:rows, :],
                                     func=ACT.Ln)
                adj = work.tile([128, 1], F32, tag="adj")
                nc.vector.tensor_sub(adj[:rows, :], R80[:rows, :], Lt[:rows, :])

                Vg = work.tile([128, 80], F32, tag="vg")
                Ig = work.tile([128, 80], F32, tag="ig")
                nc.vector.tensor_copy(
                    Vg[:rows, :80].rearrange("r (s e) -> r s e", e=K), Vv)
                nc.vector.tensor_copy(
                    Ig[:rows, :80].rearrange("r (s e) -> r s e", e=K), Iv)

                c1 = work.tile([128, 8], F32, tag="c1")
                c2 = work.tile([128, 8], F32, tag="c2")
                Vz = work.tile([128, 80], F32, tag="vz")
                nc.vector.max(c1[:rows, :], Vg[:rows, :80])
                nc.vector.match_replace(out=Vz[:rows, :80],
                                        in_to_replace=c1[:rows, :],
                                        in_values=Vg[:rows, :80], imm_value=NEG)
                nc.vector.max(c2[:rows, :], Vz[:rows, :80])

                v10 = work.tile([128, K], F32, tag="v10")
                nc.vector.tensor_copy(v10[:rows, 0:8], c1[:rows, :])
                nc.vector.tensor_copy(v10[:rows, 8:10], c2[:rows, 0:2])

                # gather token ids of the winners by value match
                tok10 = work.tile([128, K], F32, tag="tok10")
                for j in range(K):
                    nc.vector.scalar_tensor_tensor(
                        out=junkg[:rows, :80], in0=Vg[:rows, :80],
                        scalar=v10[:rows, j:j + 1], in1=Ig[:rows, :80],
                        op0=ALU.is_equal, op1=ALU.mult,
                        accum_out=tok10[:rows, j:j + 1],
                    )

                # cumulative path scores of the 10 winners (descending)
                comb10 = work.tile([128, K], F32, tag="comb10")
                nc.vector.tensor_tensor(comb10[:rows, :K], v10[:rows, :K],
                                        adj[:rows, :].to_broadcast([rows, K]),
                                        op=ALU.add)

                if l == 0:
                    # history block 0, frontier = the 10 winners per sequence
                    nc.vector.tensor_copy(hist_sc_sb[:B, 0:K], comb10[:B, :K])
                    nc.vector.tensor_copy(hist_tok_sb[:B, 0:K], tok10[:B, :K])
                    nc.vector.tensor_copy(sel_pos[:B, :K], iota_c[:B, :K])
                    nc.vector.memset(parents_sb[:B, 0:1], -1.0)
                    cur_scores_v = comb10[:B, :K]
                    cur_tokens_v = tok10[:B, :K]
                else:
                    # fold [80,10] row-major into [8, 100] per sequence
                    combb = work.tile([128, 100], F32, tag="combb")
                    tokb = work.tile([128, 100], F32, tag="tokb")
                    nc.sync.dma_start(
                        out=combb[:B, :100].rearrange("b (k e) -> b k e", e=K),
                        in_=comb10[:ROWS, :K],
                    )
                    nc.sync.dma_start(
                        out=tokb[:B, :100].rearrange("b (k e) -> b k e", e=K),
                        in_=tok10[:ROWS, :K],
                    )
                    comb_b = combb[:B, :100]
                    tok_b = tokb[:B, :100]

                    # history blocks for this layer
                    hoff = K + 100 * (l - 1)
                    nc.vector.tensor_copy(hist_sc_sb[:B, hoff:hoff + 100], comb_b)
                    nc.vector.tensor_copy(hist_tok_sb[:B, hoff:hoff + 100], tok_b)

                    # select new frontier: top-10 of the 100 comb scores
                    d1 = work.tile([128, 8], F32, tag="d1")
                    e1 = work.tile([128, 8], U32, tag="e1")
                    d2 = work.tile([128, 8], F32, tag="d2")
                    e2 = work.tile([128, 8], U32, tag="e2")
                    cz = work.tile([128, 100], F32, tag="cz")
                    nc.vector.max(d1[:B, :], comb_b)
                    nc.vector.max_index(e1[:B, :], d1[:B, :], comb_b)
                    nc.vector.match_replace(out=cz[:B, :100], in_to_replace=d1[:B, :],
                                            in_values=comb_b, imm_value=NEG)
                    nc.vector.max(d2[:B, :], cz[:B, :100])
                    nc.vector.max_index(e2[:B, :], d2[:B, :], cz[:B, :100])

                    cur_sc = work.tile([128, K], F32, tag="cursc")
                    tix = work.tile([128, K], F32, tag="tix")
                    nc.vector.tensor_copy(cur_sc[:B, 0:8], d1[:B, :])
                    nc.vector.tensor_copy(cur_sc[:B, 8:10], d2[:B, 0:2])
                    nc.vector.tensor_copy(tix[:B, 0:8], e1[:B, :])
                    nc.vector.tensor_copy(tix[:B, 8:10], e2[:B, 0:2])

                    # gather frontier tokens + parent positions
                    cur_tok = work.tile([128, K], F32, tag="curtok")
                    junk2 = work.tile([128, 100], F32, tag="junk2")
                    for j in range(K):
                        nc.vector.scalar_tensor_tensor(
                            out=junk2[:B, :100], in0=iota_c[:B, :100],
                            scalar=tix[:B, j:j + 1], in1=tok_b,
                            op0=ALU.is_equal, op1=ALU.mult,
                            accum_out=cur_tok[:B, j:j + 1],
                        )
                    # sel_exp[b, 10k+j] = sel_pos[b, k]
                    sel_exp = work.tile([128, 100], F32, tag="selexp")
                    nc.vector.tensor_copy(
                        sel_exp[:B, :100].rearrange("b (k e) -> b k e", e=K),
                        sel_pos[:B, :K].unsqueeze(2).broadcast_to([B, K, K]),
                    )
                    par10 = work.tile([128, K], F32, tag="par10")
                    for j in range(K):
                        nc.vector.scalar_tensor_tensor(
                            out=junk2[:B, :100], in0=iota_c[:B, :100],
                            scalar=tix[:B, j:j + 1],
                            in1=sel_exp[:B, :100],
                            op0=ALU.is_equal, op1=ALU.mult,
                            accum_out=par10[:B, j:j + 1],
                        )
                    nc.vector.tensor_copy(parents_sb[:B, 1 + K * (l - 1):1 + K * l],
                                          par10[:B, :K])
                    # sel_pos <- 10 + 100*(l-1) + tix
                    nc.vector.tensor_scalar(
                        out=sel_pos[:B, :K], in0=tix[:B, :K],
                        scalar1=float(K + 100 * (l - 1)), scalar2=None,
                        op0=ALU.add,
                    )
                    cur_scores_v = cur_sc[:B, :K]
                    cur_tokens_v = cur_tok[:B, :K]

                if l < L - 1:
                    # fold the new frontier for the next layer:
                    # R80[10b+k] = cur_scores[b,k];  tok_off[10b+k] = token id
                    nc.sync.dma_start(out=R80[:ROWS, :1], in_=cur_scores_v)
                    ctok_i = work.tile([128, K], I32, tag="ctoki")
                    nc.vector.tensor_copy(ctok_i[:B, :K], cur_tokens_v)
                    tok_off = work.tile([128, 1], I32, tag="tokoff")
                    nc.sync.dma_start(out=tok_off[:ROWS, :1], in_=ctok_i[:B, :K])

                    # gather the 80 embedding rows -> x [80, 1024]
                    x_sb = work.tile([128, D], F32, tag="xsb", bufs=3)
                    nc.gpsimd.indirect_dma_start(
                        out=x_sb[:ROWS, :D],
                        out_offset=None,
                        in_=embed_h[:, :],
                        in_offset=bass.IndirectOffsetOnAxis(
                            ap=tok_off[:ROWS, :1], axis=0),
                    )
                    # transpose to xT [128, 8*80] via PE
                    xT_next = work.tile([128, ROWS * 8], F32, tag="xt", bufs=3)
                    ps_tr = psum_p.tile([128, 4096], F32, tag="ps")
                    for k in range(8):
                        nc.tensor.transpose(
                            out=ps_tr[:, 512 * k:512 * k + ROWS],
                            in_=x_sb[:ROWS, 128 * k:128 * (k + 1)],
                            identity=ident[:ROWS, :ROWS],
                        )
                        nc.vector.tensor_copy(
                            xT_next[:, ROWS * k:ROWS * (k + 1)],
                            ps_tr[:, 512 * k:512 * k + ROWS])
                    xT_cur = xT_next

            # ---- outputs ----
            nc.sync.dma_start(out=hist_sc_o[:, :], in_=hist_sc_sb[:B, :310])
            nc.sync.dma_start(out=hist_tok_o[:, :], in_=hist_tok_sb[:B, :310])
            nc.sync.dma_start(out=parents_o[:, :], in_=parents_sb[:B, :31])

    nc.finalize()
    return nc


def make_in_maps(input_ids, embed, w_lm):
    """Build the 8 per-core input dicts from the full problem inputs."""
    input_ids = np.asarray(input_ids)
    embed = np.ascontiguousarray(np.asarray(embed, dtype=np.float32))
    w_lm = np.asarray(w_lm, dtype=np.float32)

    x0 = embed[input_ids]                      # [8, 1024]
    x0t = np.ascontiguousarray(
        x0.T.reshape(8, 128, B).transpose(1, 0, 2).reshape(128, 8 * B)
    ).astype(np.float32)

    in_maps = []
    for s in range(NCORES):
        in_maps.append({
            "w_sh": np.ascontiguousarray(w_lm[:, s * VS:(s + 1) * VS]),
            "embed": embed,
            "x0t": x0t,
            "offs": np.full((128, 1), s * VS, dtype=np.float32),
        })
    return in_maps


def finalize(hist_sc, hist_tok, parents):
    """Host-side final top-T resample (matches lax.top_k tie semantics)."""
    hist_sc = np.array(hist_sc)
    hist_tok = np.array(hist_tok)
    # Tie repair: the device orders each row's top-10 by raw logit, but the
    # reference orders by the (coarser) f32 log-softmax value with ties
    # broken by vocab index.  Where our comb scores collide exactly, reorder
    # the run by token id ascending, within each 10-entry history block.
    nblk = hist_sc.shape[1] // K
    for b in range(hist_sc.shape[0]):
        for blk in range(nblk):
            s = slice(blk * K, (blk + 1) * K)
            sc = hist_sc[b, s]
            tk = hist_tok[b, s]
            if np.unique(sc).size < K:
                o = np.lexsort((tk, -sc))
                hist_sc[b, s] = sc[o]
                hist_tok[b, s] = tk[o]
    order = np.argsort(-hist_sc, axis=1, kind="stable")[:, :T]
    final_sc = np.take_along_axis(hist_sc, order, axis=1).astype(np.float32)
    toks = np.take_along_axis(hist_tok, order, axis=1)
    new_draft_tokens = np.rint(toks).astype(np.int32).T
    topk_score_indices = order.astype(np.int32)
    parent_buffer = np.rint(parents).astype(np.int32)
    return new_draft_tokens, final_sc, topk_score_indices, parent_buffer


_NC_CACHE = None


def kernel(input_ids, embed, w_lm):
    global _NC_CACHE
    from concourse.bass_utils import run_bass_kernel_spmd

    if _NC_CACHE is None:
        _NC_CACHE = build_nc()
    nc = _NC_CACHE
    in_maps = make_in_maps(input_ids, embed, w_lm)
    res = run_bass_kernel_spmd(nc, in_maps, core_ids=list(range(NCORES)))
    out = res.results[0]
    return finalize(out["hist_sc"], out["hist_tok"], out["parents"])


# revision 39
# speedup vs baseline: 1.1588x; 1.0873x over previous
"""Dynamic tree drafting loop on 8 trn2 NeuronCores.

Strategy: tensor-parallel LM head over vocab (V=32000 -> 4000 per core).
Each core holds its w_lm shard in SBUF (16MB, loaded once), computes
logits for all frontier rows every layer, does an exact per-shard top-10
(DVE max8/max_index/match_replace), and AllGathers (candidate values,
global token ids, sum-of-exp) per layer.  Every core then redundantly
combines shards: global per-row top-10, log-softmax adjust, tree
bookkeeping (history scatter, frontier re-select, parent tracking),
embedding gather (indirect DMA) for the next layer.  The final
top-T=63 re-sample over the [8,310] history is done on host.
"""

import numpy as np

import concourse.bass as bass
import concourse.mybir as mybir
from concourse import bacc, tile
from concourse.masks import make_identity

F32 = mybir.dt.float32
BF16 = mybir.dt.bfloat16
U32 = mybir.dt.uint32
I32 = mybir.dt.int32
ALU = mybir.AluOpType
ACT = mybir.ActivationFunctionType
AX = mybir.AxisListType

K = 10          # dynamic_tree_max_topK
L = 4           # max_draft_len
T = 63          # max_total_draft_tokens
B, V, D = 8, 32000, 1024
NCORES = 8
VS = V // NCORES          # 4000 vocab per core
NCH = 8                   # psum chunks per shard
CH = VS // NCH            # 500
ROWS = B * K              # 80 frontier rows at layers >= 1
NEG = -1e30


def build_nc():
    nc = bacc.Bacc(trn_type="TRN2", target_bir_lowering=False, num_devices=NCORES)

    # ---- kernel I/O ----
    # LM-head shard as a bf16 hi/lo pair: w = w_hi + w_lo to ~2^-17 rel.
    # Three bf16 matmul passes (hi*hi + hi*lo + lo*hi) reproduce the f32
    # logits to ~1e-7 absolute at ~1.7x the fp32-matmul throughput.
    w_hi_h = nc.declare_dram_parameter("w_hi", [D, VS], BF16, isOutput=False)
    w_lo_h = nc.declare_dram_parameter("w_lo", [D, VS], BF16, isOutput=False)
    embed_h = nc.declare_dram_parameter("embed", [V, D], F32, isOutput=False)
    x0t_hi_h = nc.declare_dram_parameter("x0t_hi", [128, 8 * B], BF16,
                                         isOutput=False)
    x0t_lo_h = nc.declare_dram_parameter("x0t_lo", [128, 8 * B], BF16,
                                         isOutput=False)
    offs_h = nc.declare_dram_parameter("offs", [128, 1], F32, isOutput=False)
    hist_sc_o = nc.declare_dram_parameter("hist_sc", [B, 310], F32, isOutput=True)
    hist_tok_o = nc.declare_dram_parameter("hist_tok", [B, 310], F32, isOutput=True)
    parents_o = nc.declare_dram_parameter("parents", [B, 31], F32, isOutput=True)

    RG = [list(range(NCORES))]

    with tile.TileContext(nc) as tc:
        with (
            tc.tile_pool(name="const", bufs=1) as const,
            tc.tile_pool(name="wpool", bufs=1) as wpool,
            tc.tile_pool(name="work", bufs=1) as work,
            tc.tile_pool(name="psum", bufs=1, space="PSUM") as psum_p,
            tc.tile_pool(name="dram", bufs=2, space="DRAM") as dram_p,
        ):
            # ---- persistent / constant tiles ----
            iota_c = const.tile([128, 100], F32, tag="iota")
            nc.gpsimd.iota(iota_c[:, :], pattern=[[1, 100]], base=0,
                           channel_multiplier=0,
                           allow_small_or_imprecise_dtypes=True)
            # choff[10c+j] = 500*c : per-chunk vocab base within the shard
            choff = const.tile([128, 80], F32, tag="choff")
            nc.gpsimd.iota(choff[:, :], pattern=[[CH, NCH], [0, K]], base=0,
                           channel_multiplier=0,
                           allow_small_or_imprecise_dtypes=True)
            ident = const.tile([128, 128], F32, tag="ident")
            make_identity(nc, ident[:, :])
            offs_sb = const.tile([128, 1], F32, tag="offs")
            nc.sync.dma_start(out=offs_sb[:, :], in_=offs_h[:, :])
            x0t_hi = const.tile([128, 8 * B], BF16, tag="x0thi")
            nc.sync.dma_start(out=x0t_hi[:, :], in_=x0t_hi_h[:, :])
            x0t_lo = const.tile([128, 8 * B], BF16, tag="x0tlo")
            nc.sync.dma_start(out=x0t_lo[:, :], in_=x0t_lo_h[:, :])

            hist_sc_sb = const.tile([128, 310], F32, tag="hsc")
            hist_tok_sb = const.tile([128, 310], F32, tag="htok")
            parents_sb = const.tile([128, 31], F32, tag="par")
            R80 = const.tile([128, 1], F32, tag="r80")
            nc.vector.memset(R80[:, :], 0.0)
            sel_pos = const.tile([128, K], F32, tag="selpos")

            # ---- w shard -> SBUF, one 1MB DMA per 128-row k-slab ----
            w_hi = []
            w_lo = []
            for k in range(8):
                whk = wpool.tile([128, VS], BF16, tag=f"wh{k}")
                nc.sync.dma_start(out=whk[:, :],
                                  in_=w_hi_h[128 * k:128 * (k + 1), :])
                w_hi.append(whk)
                wlk = wpool.tile([128, VS], BF16, tag=f"wl{k}")
                nc.sync.dma_start(out=wlk[:, :],
                                  in_=w_lo_h[128 * k:128 * (k + 1), :])
                w_lo.append(wlk)

            xT_hi_cur = None  # bf16 [128, 80*8] for layers >= 1
            xT_lo_cur = None

            for l in range(L):
                rows = B if l == 0 else ROWS
                # ---------- matmul + per-chunk top-10, chunk-pipelined ------
                # Each 500-wide psum chunk is top-k'ed on DVE (reading PSUM
                # directly) and exp-summed on ACT while PE works on the next
                # chunk, so only the final chunk's reduction is exposed.
                ps = psum_p.tile([128, 4096], F32, tag="ps")
                sums = work.tile([128, NCH], F32, tag="sums")
                junk_exp = work.tile([128, CH], F32, tag="jexp")
                chunkv = work.tile([128, 80], F32, tag="chv")
                chunki = work.tile([128, 80], F32, tag="chi")
                chunkz = work.tile([128, CH], F32, tag="chz")
                mc1 = work.tile([128, 8], F32, tag="mc1")
                mc2 = work.tile([128, 8], F32, tag="mc2")
                ic1 = work.tile([128, 8], U32, tag="ic1")
                ic2 = work.tile([128, 8], U32, tag="ic2")
                for c in range(NCH):
                    for k in range(8):
                        if l == 0:
                            lhi = x0t_hi[:, B * k:B * (k + 1)]
                            llo = x0t_lo[:, B * k:B * (k + 1)]
                        else:
                            lhi = xT_hi_cur[:, ROWS * k:ROWS * (k + 1)]
                            llo = xT_lo_cur[:, ROWS * k:ROWS * (k + 1)]
                        out_c = ps[:rows, 512 * c:512 * c + CH]
                        rhi = w_hi[k][:, CH * c:CH * (c + 1)]
                        rlo = w_lo[k][:, CH * c:CH * (c + 1)]
                        nc.tensor.matmul(out_c, lhsT=lhi, rhs=rhi,
                                         start=(k == 0), stop=False)
                        nc.tensor.matmul(out_c, lhsT=lhi, rhs=rlo,
                                         start=False, stop=False)
                        nc.tensor.matmul(out_c, lhsT=llo, rhs=rhi,
                                         start=False, stop=(k == 7))
                    pvc = ps[:rows, 512 * c:512 * c + CH]
                    nc.scalar.activation(
                        out=junk_exp[:rows, :CH], in_=pvc, func=ACT.Exp,
                        accum_out=sums[:rows, c:c + 1],
                    )
                    nc.vector.max(mc1[:rows, :], pvc)
                    nc.vector.max_index(ic1[:rows, :], mc1[:rows, :], pvc)
                    nc.vector.match_replace(out=chunkz[:rows, :CH],
                                            in_to_replace=mc1[:rows, :],
                                            in_values=pvc, imm_value=NEG)
                    nc.vector.max(mc2[:rows, :], chunkz[:rows, :CH])
                    nc.vector.max_index(ic2[:rows, :], mc2[:rows, :],
                                        chunkz[:rows, :CH])
                    nc.vector.tensor_copy(chunkv[:rows, 10 * c:10 * c + 8],
                                          mc1[:rows, :])
                    nc.vector.tensor_copy(chunkv[:rows, 10 * c + 8:10 * c + 10],
                                          mc2[:rows, 0:2])
                    nc.vector.tensor_copy(chunki[:rows, 10 * c:10 * c + 8],
                                          ic1[:rows, :])
                    nc.vector.tensor_copy(chunki[:rows, 10 * c + 8:10 * c + 10],
                                          ic2[:rows, 0:2])

                sumexp = work.tile([128, 1], F32, tag="sumexp")
                nc.vector.reduce_sum(out=sumexp[:rows, :], in_=sums[:rows, :NCH],
                                     axis=AX.X)
                # globalize candidate ids: + 500*chunk + shard offset
                nc.vector.tensor_tensor(chunki[:rows, :80], chunki[:rows, :80],
                                        choff[:rows, :80], op=ALU.add)
                nc.vector.tensor_tensor(chunki[:rows, :80], chunki[:rows, :80],
                                        offs_sb[:rows, :].to_broadcast([rows, 80]),
                                        op=ALU.add)

                # ---------- shard-level top-10 of the 80 chunk candidates ----
                sm1 = work.tile([128, 8], F32, tag="sm1")
                sm2 = work.tile([128, 8], F32, tag="sm2")
                sz = work.tile([128, 80], F32, tag="sz")
                nc.vector.max(sm1[:rows, :], chunkv[:rows, :80])
                nc.vector.match_replace(out=sz[:rows, :80],
                                        in_to_replace=sm1[:rows, :],
                                        in_values=chunkv[:rows, :80],
                                        imm_value=NEG)
                nc.vector.max(sm2[:rows, :], sz[:rows, :80])
                v10s = work.tile([128, K], F32, tag="v10s")
                nc.vector.tensor_copy(v10s[:rows, 0:8], sm1[:rows, :])
                nc.vector.tensor_copy(v10s[:rows, 8:10], sm2[:rows, 0:2])
                tok10s = work.tile([128, K], F32, tag="tok10s")
                junkg = work.tile([128, 80], F32, tag="junkg")
                for j in range(K):
                    nc.vector.scalar_tensor_tensor(
                        out=junkg[:rows, :80], in0=chunkv[:rows, :80],
                        scalar=v10s[:rows, j:j + 1], in1=chunki[:rows, :80],
                        op0=ALU.is_equal, op1=ALU.mult,
                        accum_out=tok10s[:rows, j:j + 1],
                    )

                # ---------- candidate block [80, 24] & AllGather ----------
                blk = work.tile([128, 24], F32, tag="blk")
                nc.vector.memset(blk[:80, :24], 0.0)
                nc.vector.tensor_copy(blk[:rows, 0:K], v10s[:rows, :K])
                nc.vector.tensor_copy(blk[:rows, K:2 * K], tok10s[:rows, :K])
                nc.vector.tensor_copy(blk[:rows, 20:21], sumexp[:rows, :])

                cc_in = dram_p.tile([80, 24], F32, tag="ccin", bufs=4)
                cc_out = dram_p.tile([80 * NCORES, 24], F32, tag="ccout",
                                     addr_space="Shared", bufs=4)
                nc.sync.dma_start(out=cc_in[:80, :], in_=blk[:80, :24])
                nc.gpsimd.collective_compute(
                    "AllGather", ALU.bypass, replica_groups=RG,
                    ins=[cc_in[:, :].opt()], outs=[cc_out[:, :].opt()],
                )

                # ---------- readback + cross-shard combine (all cores) ----------
                gat = work.tile([128, 24 * NCORES], F32, tag="gat")
                gsrc = cc_out[:, :].rearrange("(s r) e -> r s e", s=NCORES)[:rows]
                nc.sync.dma_start(
                    out=gat[:rows, :24 * NCORES].rearrange("r (s e) -> r s e", e=24),
                    in_=gsrc)
                g3 = gat[:rows, :24 * NCORES].rearrange("r (s e) -> r s e", e=24)
                Vv = g3[:, :, 0:K]            # [rows, 8, 10] candidate values
                Iv = g3[:, :, K:2 * K]        # [rows, 8, 10] global token ids
                Sv = g3[:, :, 20:21]          # [rows, 8, 1] per-shard sumexp

                sumall = work.tile([128, 1], F32, tag="sumall")
                nc.vector.reduce_sum(out=sumall[:rows, :], in_=Sv, axis=AX.XY)
                Lt = work.tile([128, 1], F32, tag="lt")
                nc.scalar.activation(out=Lt[:rows, :], in_=sumall[:rows, :],
                                     func=ACT.Ln)
                adj = work.tile([128, 1], F32, tag="adj")
                nc.vector.tensor_sub(adj[:rows, :], R80[:rows, :], Lt[:rows, :])

                Vg = work.tile([128, 80], F32, tag="vg")
                Ig = work.tile([128, 80], F32, tag="ig")
                nc.vector.tensor_copy(
                    Vg[:rows, :80].rearrange("r (s e) -> r s e", e=K), Vv)
                nc.vector.tensor_copy(
                    Ig[:rows, :80].rearrange("r (s e) -> r s e", e=K), Iv)

                c1 = work.tile([128, 8], F32, tag="c1")
                c2 = work.tile([128, 8], F32, tag="c2")
                Vz = work.tile([128, 80], F32, tag="vz")
                nc.vector.max(c1[:rows, :], Vg[:rows, :80])
                nc.vector.match_replace(out=Vz[:rows, :80],
                                        in_to_replace=c1[:rows, :],
                                        in_values=Vg[:rows, :80], imm_value=NEG)
                nc.vector.max(c2[:rows, :], Vz[:rows, :80])

                v10 = work.tile([128, K], F32, tag="v10")
                nc.vector.tensor_copy(v10[:rows, 0:8], c1[:rows, :])
                nc.vector.tensor_copy(v10[:rows, 8:10], c2[:rows, 0:2])

                # gather token ids of the winners by value match
                tok10 = work.tile([128, K], F32, tag="tok10")
                for j in range(K):
                    nc.vector.scalar_tensor_tensor(
                        out=junkg[:rows, :80], in0=Vg[:rows, :80],
                        scalar=v10[:rows, j:j + 1], in1=Ig[:rows, :80],
                        op0=ALU.is_equal, op1=ALU.mult,
                        accum_out=tok10[:rows, j:j + 1],
                    )

                # cumulative path scores of the 10 winners (descending)
                comb10 = work.tile([128, K], F32, tag="comb10")
                nc.vector.tensor_tensor(comb10[:rows, :K], v10[:rows, :K],
                                        adj[:rows, :].to_broadcast([rows, K]),
                                        op=ALU.add)

                if l == 0:
                    # history block 0, frontier = the 10 winners per sequence
                    nc.vector.tensor_copy(hist_sc_sb[:B, 0:K], comb10[:B, :K])
                    nc.vector.tensor_copy(hist_tok_sb[:B, 0:K], tok10[:B, :K])
                    nc.vector.tensor_copy(sel_pos[:B, :K], iota_c[:B, :K])
                    nc.vector.memset(parents_sb[:B, 0:1], -1.0)
                    cur_scores_v = comb10[:B, :K]
                    cur_tokens_v = tok10[:B, :K]
                else:
                    # fold [80,10] row-major into [8, 100] per sequence
                    combb = work.tile([128, 100], F32, tag="combb")
                    tokb = work.tile([128, 100], F32, tag="tokb")
                    nc.sync.dma_start(
                        out=combb[:B, :100].rearrange("b (k e) -> b k e", e=K),
                        in_=comb10[:ROWS, :K],
                    )
                    nc.sync.dma_start(
                        out=tokb[:B, :100].rearrange("b (k e) -> b k e", e=K),
                        in_=tok10[:ROWS, :K],
                    )
                    comb_b = combb[:B, :100]
                    tok_b = tokb[:B, :100]

                    # history blocks for this layer
                    hoff = K + 100 * (l - 1)
                    nc.vector.tensor_copy(hist_sc_sb[:B, hoff:hoff + 100], comb_b)
                    nc.vector.tensor_copy(hist_tok_sb[:B, hoff:hoff + 100], tok_b)

                    # select new frontier: top-10 of the 100 comb scores
                    d1 = work.tile([128, 8], F32, tag="d1")
                    e1 = work.tile([128, 8], U32, tag="e1")
                    d2 = work.tile([128, 8], F32, tag="d2")
                    e2 = work.tile([128, 8], U32, tag="e2")
                    cz = work.tile([128, 100], F32, tag="cz")
                    nc.vector.max(d1[:B, :], comb_b)
                    nc.vector.max_index(e1[:B, :], d1[:B, :], comb_b)
                    nc.vector.match_replace(out=cz[:B, :100], in_to_replace=d1[:B, :],
                                            in_values=comb_b, imm_value=NEG)
                    nc.vector.max(d2[:B, :], cz[:B, :100])
                    nc.vector.max_index(e2[:B, :], d2[:B, :], cz[:B, :100])

                    cur_sc = work.tile([128, K], F32, tag="cursc")
                    tix = work.tile([128, K], F32, tag="tix")
                    nc.vector.tensor_copy(cur_sc[:B, 0:8], d1[:B, :])
                    nc.vector.tensor_copy(cur_sc[:B, 8:10], d2[:B, 0:2])
                    nc.vector.tensor_copy(tix[:B, 0:8], e1[:B, :])
                    nc.vector.tensor_copy(tix[:B, 8:10], e2[:B, 0:2])

                    # gather frontier tokens + parent positions
                    cur_tok = work.tile([128, K], F32, tag="curtok")
                    junk2 = work.tile([128, 100], F32, tag="junk2")
                    for j in range(K):
                        nc.vector.scalar_tensor_tensor(
                            out=junk2[:B, :100], in0=iota_c[:B, :100],
                            scalar=tix[:B, j:j + 1], in1=tok_b,
                            op0=ALU.is_equal, op1=ALU.mult,
                            accum_out=cur_tok[:B, j:j + 1],
                        )
                    # sel_exp[b, 10k+j] = sel_pos[b, k]
                    sel_exp = work.tile([128, 100], F32, tag="selexp")
                    nc.vector.tensor_copy(
                        sel_exp[:B, :100].rearrange("b (k e) -> b k e", e=K),
                        sel_pos[:B, :K].unsqueeze(2).broadcast_to([B, K, K]),
                    )
                    par10 = work.tile([128, K], F32, tag="par10")
                    for j in range(K):
                        nc.vector.scalar_tensor_tensor(
                            out=junk2[:B, :100], in0=iota_c[:B, :100],
                            scalar=tix[:B, j:j + 1],
                            in1=sel_exp[:B, :100],
                            op0=ALU.is_equal, op1=ALU.mult,
                            accum_out=par10[:B, j:j + 1],
                        )
                    nc.vector.tensor_copy(parents_sb[:B, 1 + K * (l - 1):1 + K * l],
                                          par10[:B, :K])
                    # sel_pos <- 10 + 100*(l-1) + tix
                    nc.vector.tensor_scalar(
                        out=sel_pos[:B, :K], in0=tix[:B, :K],
                        scalar1=float(K + 100 * (l - 1)), scalar2=None,
                        op0=ALU.add,
                    )
                    cur_scores_v = cur_sc[:B, :K]
                    cur_tokens_v = cur_tok[:B, :K]

                if l < L - 1:
                    # fold the new frontier for the next layer:
                    # R80[10b+k] = cur_scores[b,k];  tok_off[10b+k] = token id
                    nc.sync.dma_start(out=R80[:ROWS, :1], in_=cur_scores_v)
                    ctok_i = work.tile([128, K], I32, tag="ctoki")
                    nc.vector.tensor_copy(ctok_i[:B, :K], cur_tokens_v)
                    tok_off = work.tile([128, 1], I32, tag="tokoff")
                    nc.sync.dma_start(out=tok_off[:ROWS, :1], in_=ctok_i[:B, :K])

                    # gather the 80 embedding rows -> x [80, 1024]
                    x_sb = work.tile([128, D], F32, tag="xsb", bufs=3)
                    nc.gpsimd.indirect_dma_start(
                        out=x_sb[:ROWS, :D],
                        out_offset=None,
                        in_=embed_h[:, :],
                        in_offset=bass.IndirectOffsetOnAxis(
                            ap=tok_off[:ROWS, :1], axis=0),
                    )
                    # transpose to xT [128, 8*80] via PE, then split bf16 hi/lo
                    xT_next = work.tile([128, ROWS * 8], F32, tag="xt", bufs=2)
                    ps_tr = psum_p.tile([128, 4096], F32, tag="ps")
                    for k in range(8):
                        nc.tensor.transpose(
                            out=ps_tr[:, 512 * k:512 * k + ROWS],
                            in_=x_sb[:ROWS, 128 * k:128 * (k + 1)],
                            identity=ident[:ROWS, :ROWS],
                        )
                        nc.vector.tensor_copy(
                            xT_next[:, ROWS * k:ROWS * (k + 1)],
                            ps_tr[:, 512 * k:512 * k + ROWS])
                    xT_hi_n = work.tile([128, ROWS * 8], BF16, tag="xthi", bufs=2)
                    xT_lo_n = work.tile([128, ROWS * 8], BF16, tag="xtlo", bufs=2)
                    xlo_f = work.tile([128, ROWS * 8], F32, tag="xlof")
                    nc.vector.tensor_copy(xT_hi_n[:, :], xT_next[:, :])
                    nc.vector.tensor_tensor(xlo_f[:, :], xT_next[:, :],
                                            xT_hi_n[:, :], op=ALU.subtract)
                    nc.vector.tensor_copy(xT_lo_n[:, :], xlo_f[:, :])
                    xT_hi_cur = xT_hi_n
                    xT_lo_cur = xT_lo_n

            # ---- outputs ----
            nc.sync.dma_start(out=hist_sc_o[:, :], in_=hist_sc_sb[:B, :310])
            nc.sync.dma_start(out=hist_tok_o[:, :], in_=hist_tok_sb[:B, :310])
            nc.sync.dma_start(out=parents_o[:, :], in_=parents_sb[:B, :31])

    nc.finalize()
    return nc


def make_in_maps(input_ids, embed, w_lm):
    """Build the 8 per-core input dicts from the full problem inputs."""
    bf16 = mybir.dt.np(BF16)
    input_ids = np.asarray(input_ids)
    embed = np.ascontiguousarray(np.asarray(embed, dtype=np.float32))
    w_lm = np.asarray(w_lm, dtype=np.float32)

    w_hi = w_lm.astype(bf16)
    w_lo = (w_lm - w_hi.astype(np.float32)).astype(bf16)

    x0 = embed[input_ids]                      # [8, 1024]
    x0t = np.ascontiguousarray(
        x0.T.reshape(8, 128, B).transpose(1, 0, 2).reshape(128, 8 * B)
    ).astype(np.float32)
    x0t_hi = x0t.astype(bf16)
    x0t_lo = (x0t - x0t_hi.astype(np.float32)).astype(bf16)

    in_maps = []
    for s in range(NCORES):
        in_maps.append({
            "w_hi": np.ascontiguousarray(w_hi[:, s * VS:(s + 1) * VS]),
            "w_lo": np.ascontiguousarray(w_lo[:, s * VS:(s + 1) * VS]),
            "embed": embed,
            "x0t_hi": x0t_hi,
            "x0t_lo": x0t_lo,
            "offs": np.full((128, 1), s * VS, dtype=np.float32),
        })
    return in_maps


def finalize(hist_sc, hist_tok, parents):
    """Host-side final top-T resample (matches lax.top_k tie semantics)."""
    hist_sc = np.array(hist_sc)
    hist_tok = np.array(hist_tok)
    # Tie repair: the device orders each row's top-10 by raw logit, but the
    # reference orders by the (coarser) f32 log-softmax value with ties
    # broken by vocab index.  Where our comb scores collide exactly, reorder
    # the run by token id ascending, within each 10-entry history block.
    nblk = hist_sc.shape[1] // K
    for b in range(hist_sc.shape[0]):
        for blk in range(nblk):
            s = slice(blk * K, (blk + 1) * K)
            sc = hist_sc[b, s]
            tk = hist_tok[b, s]
            if np.unique(sc).size < K:
                o = np.lexsort((tk, -sc))
                hist_sc[b, s] = sc[o]
                hist_tok[b, s] = tk[o]
    order = np.argsort(-hist_sc, axis=1, kind="stable")[:, :T]
    final_sc = np.take_along_axis(hist_sc, order, axis=1).astype(np.float32)
    toks = np.take_along_axis(hist_tok, order, axis=1)
    new_draft_tokens = np.rint(toks).astype(np.int32).T
    topk_score_indices = order.astype(np.int32)
    parent_buffer = np.rint(parents).astype(np.int32)
    return new_draft_tokens, final_sc, topk_score_indices, parent_buffer


_NC_CACHE = None


def kernel(input_ids, embed, w_lm):
    global _NC_CACHE
    from concourse.bass_utils import run_bass_kernel_spmd

    if _NC_CACHE is None:
        _NC_CACHE = build_nc()
    nc = _NC_CACHE
    in_maps = make_in_maps(input_ids, embed, w_lm)
    res = run_bass_kernel_spmd(nc, in_maps, core_ids=list(range(NCORES)))
    out = res.results[0]
    return finalize(out["hist_sc"], out["hist_tok"], out["parents"])
